# revision 42
# baseline (speedup 1.0000x reference)
"""Batch-assign-probability (VQ codebook softmax) kernel for 8 Trainium2 cores.

Math: for each valid row x (D=512), over K=256 centers c_k:
    softmax_k(-||x - c_k||^2) == softmax_k(2 x.c_k - ||c_k||^2)
(the ||x||^2 term is constant over k and cancels in softmax).

Sharding: batch B=16 split across 8 cores (2 batches = 2048 valid rows per
core); the small centers table is replicated.

Precision scheme (fp16 matmuls, full PE rate):
  1-pass: logits ~= xh.ch           (xh=fp16(x), ch=fp16(2c^T))
  2-pass: pass A = xh.ch, pass B = a2.b2 with
              a2 = fp16(xl + xh/S),  b2 = fp16(ch + S*cl)
          so A+B = (1+1/S) xh.ch + xl.ch + xh.cl + O(S*xl.cl).
          The (1+1/S) surplus is removed exactly by the ACT exp()'s scale
          parameter (scale = S/(S+1)); the -||c||^2 bias is pre-divided by
          scale on host. Emulated max-abs softmax error ~4.5e-4 (S=128).
Output: unnormalized exp rows in fp16 plus their sum as a 257th column in
one DMA; the host divides in f32 and transposes back (removes the DVE
reciprocal+multiply and a separate esum DMA from the device tail).

DMA plan: all bulk transfers ride the single sync HWDGE ring, in FIFO order
ct plane(s), 3-row bias, then x row-groups — per-group completion sems pace
the PE, and group sizes are matched to trigger-issue rate (~650ns each) and
the ~1.5us DMA-receipt latency so the PE never stalls long enough to drop
the HAM clock gate. SWDGE (gpsimd) DMA is avoided entirely: concurrent
SWDGE halves the aggregate SDMA rate, and the scalar HWDGE ring starves
behind a busy sync ring. ones for the bias matmul is memset on device. A
dummy activation at program start forces the 1.3us ACT exp-table load into
the preamble shadow; exps run on ACT, row-maxes and exp-sums on DVE.
"""

import numpy as np

import concourse.bacc as bacc
import concourse.tile as tile
from concourse import mybir
from concourse.bass_utils import run_bass_kernel_spmd

B, T, W, C, K = 16, 2048, 512, 1, 256
VALID_T = 1024
D = W * C                       # 512
N_CORES = 8
B_PER_CORE = B // N_CORES       # 2
ROWS = B_PER_CORE * VALID_T     # 2048 rows per core
P = 128
D_CHUNKS = D // P               # 4
GROUPS = [128, 384, 512, 512, 512]    # rows per x/out DMA group
N_WARM_MM = 5                  # N=512 dummy matmuls against the HAM gate
KO = K + 1                     # out row: 256 exp values + their sum

N_PASSES = 1                   # 1 = xh.ch only; 2 = + composite correction
S_COMP = 128.0                 # composite split scale
SCALE = S_COMP / (S_COMP + 1.0) if N_PASSES == 2 else 1.0

assert sum(GROUPS) == ROWS
T_TILES = ROWS // P                        # 16
X_TOTAL = P * N_PASSES * D_CHUNKS * ROWS   # flat fp16 element count of x param
BIAS_ROWS = 3

F16_NP = np.float16

_CACHE: dict = {}


def _build_bass():
    f32 = mybir.dt.float32
    f16 = mybir.dt.float16
    H = N_PASSES
    nc = bacc.Bacc()
    # x planes (hi, composite), group-major, fully contiguous per group:
    # for each group g (R rows), block [128p, H, 4c, R] flattened.
    xp = nc.declare_dram_parameter("xp", [X_TOTAL], f16, isOutput=False)
    # ct plane blocks, each [128p, 4c, 256k] contiguous.
    ctp = nc.declare_dram_parameter("ctp", [H * P * D_CHUNKS * K], f16,
                                    isOutput=False)
    biasp = nc.declare_dram_parameter("biasp", [BIAS_ROWS, K], f16,
                                      isOutput=False)
    # out[p, t*KO + k] = exp(logit - max) for row = t*128 + p, k < K
    # (unnormalized); k == K holds sum_k exp. Host divides + transposes.
    out = nc.declare_dram_parameter("out", [P, T_TILES * KO], f16,
                                    isOutput=True)

    out_v = out.rearrange("p (t k) -> p t k", k=KO)      # [128, 16, 257]
    ct_plane = P * D_CHUNKS * K

    with tile.TileContext(nc) as tc:
        with (
            tc.tile_pool(name="singles", bufs=1) as singles,
            tc.tile_pool(name="xpool", bufs=1) as xpool,
            tc.tile_pool(name="opool", bufs=3) as opool,
            tc.tile_pool(name="small", bufs=8) as small,
            tc.tile_pool(name="psum", bufs=7, space="PSUM") as psum,
            tc.tile_pool(name="psum_warm", bufs=1, space="PSUM") as psum_warm,
        ):
            ct_sb = singles.tile([P, H, D_CHUNKS, K], f16)
            bias_sb = singles.tile([P, K], f16)
            ones_sb = singles.tile([P, P], f16)
            warm_sb = singles.tile([P, 512], f16)
            dummy_sb = singles.tile([P, 2], f32)
            # device-made constants (no DMA): ones for the bias matmul,
            # zeros below the 3 real bias rows, warmup scratch
            nc.gpsimd.memset(warm_sb[:], 0.0)
            nc.gpsimd.memset(ones_sb[:], 1.0)
            nc.gpsimd.memset(bias_sb[:], 0.0)
            # dummy ops at the head of the scalar/vector queues: the first
            # forces the ACT exp-table load to happen during the preamble
            # (otherwise it lands behind the first reduce and adds 1.3us to
            # the first real exp); the second warms the DVE path.
            nc.vector.memset(dummy_sb[:], 0.0)
            nc.scalar.activation(
                out=dummy_sb[:, :1], in_=dummy_sb[:, 1:],
                func=mybir.ActivationFunctionType.Exp,
            )

            xgs = []
            xoff = 0

            def x_dma(g, R):
                xg = xpool.tile([P, H, D_CHUNKS, R], f16, tag=f"xg{g}")
                n = P * H * D_CHUNKS * R
                src = xp[xoff:xoff + n].rearrange(
                    "(p h c r) -> p h c r", p=P, h=H, c=D_CHUNKS)
                nc.sync.dma_start(out=xg[:], in_=src)
                xgs.append(xg)
                return n

            def ct_dma(h):
                nc.sync.dma_start(
                    out=ct_sb[:, h],
                    in_=ctp[h * ct_plane:(h + 1) * ct_plane].rearrange(
                        "(p c k) -> p c k", p=P, c=D_CHUNKS),
                )

            # Everything bulk rides the sync HWDGE ring (SWDGE concurrent
            # with HWDGE halves aggregate DMA rate; the scalar ring starves
            # behind a busy sync ring). FIFO: ct plane 0 whole (splitting it
            # puts the ~1.5us DMA-receipt latency inside g0's matmul chain),
            # bias(1.5KB), then x groups — per-group sems pace the PE.
            ct_dma(0)
            nc.sync.dma_start(out=bias_sb[:BIAS_ROWS, :], in_=biasp[:])
            xoff += x_dma(0, GROUPS[0])
            if H == 2:
                ct_dma(1)
            for g, R in enumerate(GROUPS[1:], start=1):
                xoff += x_dma(g, R)

            # PE warm-up: dummy matmuls on scratch data keep the PE busy
            # through the HAM activity window while the first x DMA lands.
            warm_ps = psum_warm.tile([P, 512], f32, tag="warm")
            for _ in range(N_WARM_MM):
                nc.tensor.matmul(
                    warm_ps[:], lhsT=warm_sb[:, :P], rhs=warm_sb[:],
                    start=True, stop=True,
                )

            t0 = 0  # running 128-row tile index
            for g, R in enumerate(GROUPS):
                xg = xgs[g]
                subtiles = R // P
                og = opool.tile([P, subtiles, KO], f16, tag="og")
                for s0 in range(0, subtiles, 2):
                    pair = min(2, subtiles - s0)
                    ps = psum.tile([P, pair, K], f32, tag="ps")
                    # NOTE: keep each subtile's accumulation group contiguous
                    # — a start=True clears has_written for the whole bank,
                    # so interleaving two in-flight groups in one bank breaks
                    # the first one's accumulation.
                    for j in range(pair):
                        rsl = slice((s0 + j) * P, (s0 + j + 1) * P)
                        first = True
                        for h in range(H):
                            for c in range(D_CHUNKS):
                                nc.tensor.matmul(
                                    ps[:, j, :],
                                    lhsT=xg[:, h, c, rsl],
                                    rhs=ct_sb[:, h, c, :],
                                    start=first,
                                    stop=False,
                                )
                                first = False
                        nc.tensor.matmul(
                            ps[:, j, :],
                            lhsT=ones_sb[:],
                            rhs=bias_sb[:],
                            start=False,
                            stop=True,
                        )
                    negm = small.tile([P, pair], f32, tag="negm")
                    nc.vector.reduce_max(
                        out=negm[:], in_=ps[:], axis=mybir.AxisListType.X,
                        negate=True,
                    )
                    if SCALE != 1.0:
                        nc.vector.tensor_scalar_mul(negm[:], negm[:], SCALE)
                    for j in range(pair):
                        nc.scalar.activation(
                            out=og[:, s0 + j, :K],
                            in_=ps[:, j, :],
                            func=mybir.ActivationFunctionType.Exp,
                            bias=negm[:, j:j + 1],
                            scale=SCALE,
                        )
                    # exp row-sums into og column K on DVE (an ACT accum_out
                    # would add a 283ns READ_ACCUMULATOR per exp on the
                    # scalar queue; a separate esum DMA would lengthen the
                    # tail by a trigger + receipt)
                    with nc.allow_low_precision(
                            reason="fp16 exp-sum write; f32 internal accum, "
                                   "host divides in f32 (err ~5e-4 rel)"):
                        nc.vector.reduce_sum(
                            out=og[:, s0:s0 + pair, K:],
                            in_=og[:, s0:s0 + pair, :K],
                            axis=mybir.AxisListType.X,
                        )
                    if g == len(GROUPS) - 1:
                        # flush the final group per pair: the first pair's
                        # bytes fly while the last pair's softmax finishes,
                        # and the kernel-ending DMA halves
                        nc.sync.dma_start(
                            out=out_v[:, t0 + s0:t0 + s0 + pair, :],
                            in_=og[:, s0:s0 + pair, :],
                        )
                # all outs ride sync behind the x FIFO (an out trigger on the
                # scalar queue would block later exps behind its og wait);
                # they drain at full rate once x is done.
                if g != len(GROUPS) - 1:
                    nc.sync.dma_start(
                        out=out_v[:, t0:t0 + subtiles, :], in_=og[:])
                t0 += subtiles
    nc.finalize()
    return nc


def get_nc():
    if "nc" not in _CACHE:
        _CACHE["nc"] = _build_bass()
    return _CACHE["nc"]


def prep_inputs(y_pred: np.ndarray, mask: np.ndarray, centers: np.ndarray):
    """Host-side shard prep: valid-timestep slice, per-core transpose,
    fp16 (+ composite) planes, contiguous per-DMA packing."""
    x = np.ascontiguousarray(y_pred.reshape(B, T, D))
    masktime = np.asarray(mask).reshape(B, T, D)[0, :, 0]
    valid_idx = np.nonzero(masktime == 0)[0][:VALID_T]
    assert valid_idx.shape[0] == VALID_T
    if valid_idx[0] == 0 and valid_idx[-1] == VALID_T - 1:
        xv = x[:, :VALID_T]                    # [B, VALID_T, D]
    else:
        xv = x[:, valid_idx]

    centers = np.asarray(centers, dtype=np.float32)
    ct = (2.0 * centers).T.astype(np.float32)               # [D, K]
    ch = ct.astype(F16_NP)
    planes = [ch]
    if N_PASSES == 2:
        cl = ct - ch.astype(np.float32)
        planes.append((ch.astype(np.float32) + S_COMP * cl).astype(F16_NP))
    # [c, p, k] -> [p, c, k] contiguous per plane
    ct_blocks = [
        np.ascontiguousarray(h.reshape(D_CHUNKS, P, K).transpose(1, 0, 2)).ravel()
        for h in planes
    ]
    ctp = np.ascontiguousarray(np.concatenate(ct_blocks))

    negc2 = -(centers.astype(np.float64) ** 2).sum(axis=1)  # [K]
    bias_pre = negc2 / SCALE
    biasp = np.zeros((BIAS_ROWS, K), dtype=F16_NP)
    rem = bias_pre
    for i in range(BIAS_ROWS):
        biasp[i] = rem.astype(F16_NP)
        rem = rem - biasp[i].astype(np.float64)

    in_maps = []
    for core in range(N_CORES):
        xc = xv[core * B_PER_CORE:(core + 1) * B_PER_CORE].reshape(ROWS, D)
        xTc = np.ascontiguousarray(xc.T).astype(np.float32)  # [D, ROWS]
        xh = xTc.astype(F16_NP)
        xplanes = [xh]
        if N_PASSES == 2:
            xl = xTc - xh.astype(np.float32)
            xplanes.append(
                (xl + xh.astype(np.float32) / S_COMP).astype(F16_NP))
        # [h, c, p, row] -> [p, h, c, row]
        base = np.stack(xplanes).reshape(
            N_PASSES, D_CHUNKS, P, ROWS).transpose(2, 0, 1, 3)
        blocks = []
        r0 = 0
        for R in GROUPS:
            blocks.append(np.ascontiguousarray(base[:, :, :, r0:r0 + R]).ravel())
            r0 += R
        xp = np.concatenate(blocks)
        assert xp.shape[0] == X_TOTAL
        in_maps.append({"xp": xp, "ctp": ctp, "biasp": biasp})
    return in_maps


def kernel(y_pred: np.ndarray, mask: np.ndarray, centers: np.ndarray,
           **run_kwargs) -> np.ndarray:
    in_maps = prep_inputs(y_pred, mask, centers)
    nc = get_nc()
    last_err = None
    for _attempt in range(3):
        try:
            res = run_bass_kernel_spmd(nc, in_maps, core_ids=list(range(N_CORES)),
                                       **run_kwargs)
            break
        except Exception as e:  # transient NRT device errors — retry
            last_err = e
    else:
        raise last_err
    _CACHE["last_results"] = res
    outs = []
    for r in res.results:
        e = r["out"].astype(np.float32).reshape(P, T_TILES, KO)
        p = e[:, :, :K] / e[:, :, K:]
        outs.append(p.transpose(1, 0, 2).reshape(B_PER_CORE, VALID_T, K))
    return np.concatenate(outs, axis=0)


# revision 43
# speedup vs baseline: 1.0208x; 1.0208x over previous
"""Batch-assign-probability (VQ codebook softmax) kernel for 8 Trainium2 cores.

Math: for each valid row x (D=512), over K=256 centers c_k:
    softmax_k(-||x - c_k||^2) == softmax_k(2 x.c_k - ||c_k||^2)
(the ||x||^2 term is constant over k and cancels in softmax).

Sharding: batch B=16 split across 8 cores (2 batches = 2048 valid rows per
core); the small centers table is replicated.

Precision scheme (fp16 matmuls, full PE rate):
  1-pass: logits ~= xh.ch           (xh=fp16(x), ch=fp16(2c^T))
  2-pass: pass A = xh.ch, pass B = a2.b2 with
              a2 = fp16(xl + xh/S),  b2 = fp16(ch + S*cl)
          so A+B = (1+1/S) xh.ch + xl.ch + xh.cl + O(S*xl.cl).
          The (1+1/S) surplus is removed exactly by the ACT exp()'s scale
          parameter (scale = S/(S+1)); the -||c||^2 bias is pre-divided by
          scale on host. Emulated max-abs softmax error ~4.5e-4 (S=128).
Output: unnormalized exp rows in fp16 plus their sum as a 257th column in
one DMA; the host divides in f32 and transposes back (removes the DVE
reciprocal+multiply and a separate esum DMA from the device tail).

DMA plan: all bulk transfers ride the single sync HWDGE ring, in FIFO order
ct plane(s), 3-row bias, then x row-groups — per-group completion sems pace
the PE, and group sizes are matched to trigger-issue rate (~650ns each) and
the ~1.5us DMA-receipt latency so the PE never stalls long enough to drop
the HAM clock gate. SWDGE (gpsimd) DMA is avoided entirely: concurrent
SWDGE halves the aggregate SDMA rate, and the scalar HWDGE ring starves
behind a busy sync ring. ones for the bias matmul is memset on device. A
dummy activation at program start forces the 1.3us ACT exp-table load into
the preamble shadow; exps run on ACT, row-maxes and exp-sums on DVE.
"""

import numpy as np

import concourse.bacc as bacc
import concourse.tile as tile
from concourse import mybir
from concourse.bass_utils import run_bass_kernel_spmd

B, T, W, C, K = 16, 2048, 512, 1, 256
VALID_T = 1024
D = W * C                       # 512
N_CORES = 8
B_PER_CORE = B // N_CORES       # 2
ROWS = B_PER_CORE * VALID_T     # 2048 rows per core
P = 128
D_CHUNKS = D // P               # 4
GROUPS = [128, 384, 512, 512, 512]    # rows per x/out DMA group
N_WARM_MM = 5                  # N=512 dummy matmuls against the HAM gate
KO = K + 1                     # out row: 256 exp values + their sum

N_PASSES = 1                   # 1 = xh.ch only; 2 = + composite correction
S_COMP = 128.0                 # composite split scale
SCALE = S_COMP / (S_COMP + 1.0) if N_PASSES == 2 else 1.0

assert sum(GROUPS) == ROWS
T_TILES = ROWS // P                        # 16
X_TOTAL = P * N_PASSES * D_CHUNKS * ROWS   # flat fp16 element count of x param
BIAS_ROWS = 3

F16_NP = np.float16

_CACHE: dict = {}


def _build_bass():
    f32 = mybir.dt.float32
    f16 = mybir.dt.float16
    H = N_PASSES
    nc = bacc.Bacc()
    # x planes (hi, composite), group-major, fully contiguous per group:
    # for each group g (R rows), block [128p, H, 4c, R] flattened.
    xp = nc.declare_dram_parameter("xp", [X_TOTAL], f16, isOutput=False)
    # ct plane blocks, each [128p, 4c, 256k] contiguous.
    ctp = nc.declare_dram_parameter("ctp", [H * P * D_CHUNKS * K], f16,
                                    isOutput=False)
    biasp = nc.declare_dram_parameter("biasp", [BIAS_ROWS, K], f16,
                                      isOutput=False)
    # out[p, t*KO + k] = exp(logit - max) for row = t*128 + p, k < K
    # (unnormalized); k == K holds sum_k exp. Host divides + transposes.
    out = nc.declare_dram_parameter("out", [P, T_TILES * KO], f16,
                                    isOutput=True)

    out_v = out.rearrange("p (t k) -> p t k", k=KO)      # [128, 16, 257]
    ct_plane = P * D_CHUNKS * K

    with tile.TileContext(nc) as tc:
        with (
            tc.tile_pool(name="singles", bufs=1) as singles,
            tc.tile_pool(name="xpool", bufs=1) as xpool,
            tc.tile_pool(name="opool", bufs=3) as opool,
            tc.tile_pool(name="small", bufs=8) as small,
            tc.tile_pool(name="psum", bufs=7, space="PSUM") as psum,
            tc.tile_pool(name="psum_warm", bufs=1, space="PSUM") as psum_warm,
        ):
            ct_sb = singles.tile([P, H, D_CHUNKS, K], f16)
            bias_sb = singles.tile([P, K], f16)
            ones_sb = singles.tile([P, P], f16)
            warm_sb = singles.tile([P, 512], f16)
            dummy_sb = singles.tile([P, 2], f32)
            # device-made constants (no DMA): ones for the bias matmul,
            # zeros below the 3 real bias rows, warmup scratch
            nc.gpsimd.memset(warm_sb[:], 0.0)
            nc.gpsimd.memset(ones_sb[:], 1.0)
            nc.gpsimd.memset(bias_sb[:], 0.0)
            # dummy ops at the head of the scalar/vector queues: the first
            # forces the ACT exp-table load to happen during the preamble
            # (otherwise it lands behind the first reduce and adds 1.3us to
            # the first real exp); the second warms the DVE path.
            nc.vector.memset(dummy_sb[:], 0.0)
            nc.scalar.activation(
                out=dummy_sb[:, :1], in_=dummy_sb[:, 1:],
                func=mybir.ActivationFunctionType.Exp,
            )

            xgs = []
            xoff = 0

            def x_dma(g, R):
                xg = xpool.tile([P, H, D_CHUNKS, R], f16, tag=f"xg{g}")
                n = P * H * D_CHUNKS * R
                src = xp[xoff:xoff + n].rearrange(
                    "(p h c r) -> p h c r", p=P, h=H, c=D_CHUNKS)
                nc.sync.dma_start(out=xg[:], in_=src)
                xgs.append(xg)
                return n

            def ct_dma(h):
                nc.sync.dma_start(
                    out=ct_sb[:, h],
                    in_=ctp[h * ct_plane:(h + 1) * ct_plane].rearrange(
                        "(p c k) -> p c k", p=P, c=D_CHUNKS),
                )

            # Everything bulk rides the sync HWDGE ring (SWDGE concurrent
            # with HWDGE halves aggregate DMA rate; the scalar ring starves
            # behind a busy sync ring). FIFO: ct plane 0 whole (splitting it
            # puts the ~1.5us DMA-receipt latency inside g0's matmul chain),
            # bias(1.5KB), then x groups — per-group sems pace the PE.
            ct_dma(0)
            nc.sync.dma_start(out=bias_sb[:BIAS_ROWS, :], in_=biasp[:])
            xoff += x_dma(0, GROUPS[0])
            if H == 2:
                ct_dma(1)
            for g, R in enumerate(GROUPS[1:], start=1):
                xoff += x_dma(g, R)

            # PE warm-up: dummy matmuls on scratch data keep the PE busy
            # through the HAM activity window while the first x DMA lands.
            warm_ps = psum_warm.tile([P, 512], f32, tag="warm")
            for _ in range(N_WARM_MM):
                nc.tensor.matmul(
                    warm_ps[:], lhsT=warm_sb[:, :P], rhs=warm_sb[:],
                    start=True, stop=True,
                )

            t0 = 0  # running 128-row tile index
            for g, R in enumerate(GROUPS):
                xg = xgs[g]
                subtiles = R // P
                og = opool.tile([P, subtiles, KO], f16, tag="og")
                for s0 in range(0, subtiles, 2):
                    pair = min(2, subtiles - s0)
                    ps = psum.tile([P, pair, K], f32, tag="ps")
                    # NOTE: keep each subtile's accumulation group contiguous
                    # — a start=True clears has_written for the whole bank,
                    # so interleaving two in-flight groups in one bank breaks
                    # the first one's accumulation.
                    for j in range(pair):
                        rsl = slice((s0 + j) * P, (s0 + j + 1) * P)
                        first = True
                        for h in range(H):
                            for c in range(D_CHUNKS):
                                nc.tensor.matmul(
                                    ps[:, j, :],
                                    lhsT=xg[:, h, c, rsl],
                                    rhs=ct_sb[:, h, c, :],
                                    start=first,
                                    stop=False,
                                )
                                first = False
                        nc.tensor.matmul(
                            ps[:, j, :],
                            lhsT=ones_sb[:],
                            rhs=bias_sb[:],
                            start=False,
                            stop=True,
                        )
                    negm = small.tile([P, pair], f32, tag="negm")
                    nc.vector.reduce_max(
                        out=negm[:], in_=ps[:], axis=mybir.AxisListType.X,
                        negate=True,
                    )
                    if SCALE != 1.0:
                        nc.vector.tensor_scalar_mul(negm[:], negm[:], SCALE)
                    for j in range(pair):
                        nc.scalar.activation(
                            out=og[:, s0 + j, :K],
                            in_=ps[:, j, :],
                            func=mybir.ActivationFunctionType.Exp,
                            bias=negm[:, j:j + 1],
                            scale=SCALE,
                        )
                    # exp row-sums into og column K on DVE (an ACT accum_out
                    # would add a 283ns READ_ACCUMULATOR per exp on the
                    # scalar queue; a separate esum DMA would lengthen the
                    # tail by a trigger + receipt)
                    with nc.allow_low_precision(
                            reason="fp16 exp-sum write; f32 internal accum, "
                                   "host divides in f32 (err ~5e-4 rel)"):
                        nc.vector.reduce_sum(
                            out=og[:, s0:s0 + pair, K:],
                            in_=og[:, s0:s0 + pair, :K],
                            axis=mybir.AxisListType.X,
                        )
                # all outs ride sync behind the x FIFO (an out trigger on the
                # scalar queue would block later exps behind its og wait);
                # they drain at full rate once x is done.
                nc.sync.dma_start(out=out_v[:, t0:t0 + subtiles, :], in_=og[:])
                t0 += subtiles
    nc.finalize()
    return nc


def get_nc():
    if "nc" not in _CACHE:
        _CACHE["nc"] = _build_bass()
    return _CACHE["nc"]


def prep_inputs(y_pred: np.ndarray, mask: np.ndarray, centers: np.ndarray):
    """Host-side shard prep: valid-timestep slice, per-core transpose,
    fp16 (+ composite) planes, contiguous per-DMA packing."""
    x = np.ascontiguousarray(y_pred.reshape(B, T, D))
    masktime = np.asarray(mask).reshape(B, T, D)[0, :, 0]
    valid_idx = np.nonzero(masktime == 0)[0][:VALID_T]
    assert valid_idx.shape[0] == VALID_T
    if valid_idx[0] == 0 and valid_idx[-1] == VALID_T - 1:
        xv = x[:, :VALID_T]                    # [B, VALID_T, D]
    else:
        xv = x[:, valid_idx]

    centers = np.asarray(centers, dtype=np.float32)
    ct = (2.0 * centers).T.astype(np.float32)               # [D, K]
    ch = ct.astype(F16_NP)
    planes = [ch]
    if N_PASSES == 2:
        cl = ct - ch.astype(np.float32)
        planes.append((ch.astype(np.float32) + S_COMP * cl).astype(F16_NP))
    # [c, p, k] -> [p, c, k] contiguous per plane
    ct_blocks = [
        np.ascontiguousarray(h.reshape(D_CHUNKS, P, K).transpose(1, 0, 2)).ravel()
        for h in planes
    ]
    ctp = np.ascontiguousarray(np.concatenate(ct_blocks))

    negc2 = -(centers.astype(np.float64) ** 2).sum(axis=1)  # [K]
    bias_pre = negc2 / SCALE
    biasp = np.zeros((BIAS_ROWS, K), dtype=F16_NP)
    rem = bias_pre
    for i in range(BIAS_ROWS):
        biasp[i] = rem.astype(F16_NP)
        rem = rem - biasp[i].astype(np.float64)

    in_maps = []
    for core in range(N_CORES):
        xc = xv[core * B_PER_CORE:(core + 1) * B_PER_CORE].reshape(ROWS, D)
        xTc = np.ascontiguousarray(xc.T).astype(np.float32)  # [D, ROWS]
        xh = xTc.astype(F16_NP)
        xplanes = [xh]
        if N_PASSES == 2:
            xl = xTc - xh.astype(np.float32)
            xplanes.append(
                (xl + xh.astype(np.float32) / S_COMP).astype(F16_NP))
        # [h, c, p, row] -> [p, h, c, row]
        base = np.stack(xplanes).reshape(
            N_PASSES, D_CHUNKS, P, ROWS).transpose(2, 0, 1, 3)
        blocks = []
        r0 = 0
        for R in GROUPS:
            blocks.append(np.ascontiguousarray(base[:, :, :, r0:r0 + R]).ravel())
            r0 += R
        xp = np.concatenate(blocks)
        assert xp.shape[0] == X_TOTAL
        in_maps.append({"xp": xp, "ctp": ctp, "biasp": biasp})
    return in_maps


def kernel(y_pred: np.ndarray, mask: np.ndarray, centers: np.ndarray,
           **run_kwargs) -> np.ndarray:
    in_maps = prep_inputs(y_pred, mask, centers)
    nc = get_nc()
    last_err = None
    for _attempt in range(3):
        try:
            res = run_bass_kernel_spmd(nc, in_maps, core_ids=list(range(N_CORES)),
                                       **run_kwargs)
            break
        except Exception as e:  # transient NRT device errors — retry
            last_err = e
    else:
        raise last_err
    _CACHE["last_results"] = res
    outs = []
    for r in res.results:
        e = r["out"].astype(np.float32).reshape(P, T_TILES, KO)
        p = e[:, :, :K] / e[:, :, K:]
        outs.append(p.transpose(1, 0, 2).reshape(B_PER_CORE, VALID_T, K))
    return np.concatenate(outs, axis=0)
